# revision 6
# baseline (speedup 1.0000x reference)
"""Trainium2 Bass kernel for the AUV Fossen dynamics RK2 step (nn_AUVFossen).

Per row (batch K): x[13] = pos(3)+quat(x,y,z,w)(4)+v(6), u[6].
  k1 = f(x,u); k2 = f(x + DT*k1, u); out = normquat(x + DT/2*(k1+k2))

Sharding: pure data parallel over 8 NeuronCores (batch split).
Layout: batch-major. Each core holds its rows as SBUF tiles [128, C*W]
(W consecutive rows per partition, components interleaved w*C+c). All
compute is elementwise DVE/ACT ops on strided [128, W(,n)] APs; per-row
linear algebra is unrolled into grouped component ops.

For the structured parameters this problem ships:
  -Dv = linDamp@v + qd_i|v_i|v_i ; -Cv = [-m1*(va x vl); 0]
  -g  = [c1*rot2 ; c2 x rot2]  (rot2 = third row of rot)
  vDot = rhs / m_i  (mTot ~ diag(m1,m1,m1,m2,m2,m2))
"""

import os
import sys

for _p in ("/opt/trn_rl_repo", "/root/.axon_site/_ro/trn_rl_repo"):
    if os.path.isdir(_p) and _p not in sys.path:
        sys.path.insert(0, _p)

import numpy as np

import concourse.bacc as bacc
import concourse.bass as bass
import concourse.mybir as mybir
from concourse.alu_op_type import AluOpType
from concourse.tile import TileContext

F32 = mybir.dt.float32
AF = mybir.ActivationFunctionType
MULT = AluOpType.mult
ADD = AluOpType.add
SUB = AluOpType.subtract

DT = 0.1
GRAVITY = 9.81
DENSITY = 1028.0

N_CORES = 8
P = 128

TRACE = False          # set by test.py
LAST_RUN_INFO = {}


class CompView:
    """Component view of a [128, C*W] tile region with per-row interleave
    (w-outer, comp-inner).  ap(c0, n, cstep) -> AP of shape [128, W, n]."""

    def __init__(self, tile_ap, wstride, base, W):
        self.t = tile_ap.tensor
        self.off = tile_ap.offset + base
        self.part = tile_ap.ap[0]
        self.ws = wstride
        self.W = W

    def ap(self, c0, n=1, cstep=1):
        if n == 1:
            return bass.AP(self.t, self.off + c0, [self.part, [self.ws, self.W]])
        return bass.AP(self.t, self.off + c0,
                       [self.part, [self.ws, self.W], [cstep, n]])

    def bcast(self, c0, n):
        return bass.AP(self.t, self.off + c0,
                       [self.part, [self.ws, self.W], [0, n]])


class Region:
    """Contiguous run of planar scratch slots (slot = [128, W] plane)."""

    def __init__(self, scr, slot0, n):
        self.scr = scr
        self.slot0 = slot0
        self.n = n

    def ap(self, s0=0, n=1, sstep=1):
        scr = self.scr
        off = scr.off + (self.slot0 + s0) * scr.W
        if n == 1:
            return bass.AP(scr.t, off, [scr.part, [1, scr.W]])
        return bass.AP(scr.t, off,
                       [scr.part, [1, scr.W], [sstep * scr.W, n]])

    def bcast(self, s0, n):
        scr = self.scr
        off = scr.off + (self.slot0 + s0) * scr.W
        return bass.AP(scr.t, off, [scr.part, [1, scr.W], [0, n]])


class PlanarView:
    """CompView-compatible interface over a scratch Region (comp = slot)."""

    def __init__(self, reg):
        self.reg = reg

    def ap(self, c0, n=1, cstep=1):
        return self.reg.ap(c0, n, cstep)

    def bcast(self, c0, n):
        return self.reg.bcast(c0, n)


class Scratch:
    def __init__(self, tile_ap, nslots, W):
        self.t = tile_ap.tensor
        self.off = tile_ap.offset
        self.part = tile_ap.ap[0]
        self.W = W
        self.free_slots = set(range(nslots))
        self.regions = {}

    def alloc(self, name, n):
        fs = sorted(self.free_slots)
        run = None
        for i in range(len(fs) - n + 1):
            if fs[i + n - 1] - fs[i] == n - 1:
                run = fs[i]
                break
        assert run is not None, f"scratch OOM for {name}({n}); free={len(fs)}"
        for s in range(run, run + n):
            self.free_slots.remove(s)
        self.regions[name] = (run, n)
        return Region(self, run, n)

    def free(self, *names):
        for name in names:
            run, n = self.regions.pop(name)
            self.free_slots.update(range(run, run + n))


def _extract_params(inputs):
    mass = float(np.asarray(inputs["mass"]).reshape(-1)[0])
    volume = float(np.asarray(inputs["volume"]).reshape(-1)[0])
    cog = np.asarray(inputs["cog"], np.float64).reshape(3)
    cob = np.asarray(inputs["cob"], np.float64).reshape(3)
    mTot = np.asarray(inputs["mTot"], np.float64).reshape(6, 6)
    linDamp = np.asarray(inputs["linDamp"], np.float64).reshape(6, 6)
    linDampFow = np.asarray(inputs["linDampFow"], np.float64).reshape(6, 6)
    quadDamp = np.asarray(inputs["quadDamp"], np.float64).reshape(6, 6)

    scale = max(np.abs(mTot).max(), 1e-30)
    tl, tr = mTot[0:3, 0:3], mTot[0:3, 3:6]
    bl, br = mTot[3:6, 0:3], mTot[3:6, 3:6]
    m1 = float(np.trace(tl) / 3.0)
    m2 = float(np.trace(br) / 3.0)
    structured = (
        np.abs(tl - m1 * np.eye(3)).max() < 1e-5 * scale
        and np.abs(br - m2 * np.eye(3)).max() < 1e-5 * scale
        and np.abs(tr).max() < 1e-5 * scale
        and np.abs(bl).max() < 1e-5 * scale
    )
    if not structured:
        raise NotImplementedError("unstructured mTot not supported")
    if np.abs(linDampFow).max() > 1e-30:
        raise NotImplementedError("nonzero linDampFow not supported")

    minv = np.diag(np.linalg.inv(mTot))
    ld = linDamp.copy()                 # -Dv_lin = +linDamp @ v
    qd = np.diag(quadDamp)              # only diag of quadDamp matters
    c1 = GRAVITY * (volume * DENSITY - mass)
    c2 = -mass * GRAVITY * cog + volume * DENSITY * GRAVITY * cob
    return dict(m1=m1, minv=minv, ld=ld, qd=qd, c1=float(c1), c2=c2)


def build_program(pp, K_core, W):
    assert K_core % (P * W) == 0
    n_chunks = K_core // (P * W)

    nc = bacc.Bacc("TRN2", target_bir_lowering=False, debug=False,
                   num_devices=N_CORES)
    x_d = nc.dram_tensor("x", (K_core, 13), F32, kind="ExternalInput")
    u_d = nc.dram_tensor("u", (K_core, 6), F32, kind="ExternalInput")
    o_d = nc.dram_tensor("o", (K_core, 13), F32, kind="ExternalOutput")

    xr = x_d.rearrange("(n p w) c -> n p (w c)", p=P, w=W)
    ur = u_d.rearrange("(n p w) c -> n p (w c)", p=P, w=W)
    orr = o_d.rearrange("(n p w) c -> n p (w c)", p=P, w=W)

    with TileContext(nc) as tc:
        with tc.tile_pool(name="main", bufs=2 if n_chunks > 1 else 1) as pool:
            for ci in range(n_chunks):
                X = pool.tile([P, 13 * W], F32, tag="X")
                U = pool.tile([P, 6 * W], F32, tag="U")
                O = pool.tile([P, 13 * W], F32, tag="O")
                SCRT = pool.tile([P, 62 * W], F32, tag="SCR")
                nc.sync.dma_start(X[:, :], xr[ci])
                nc.sync.dma_start(U[:, :], ur[ci])
                emit_chunk(nc, pp, X[:, :], U[:, :], O[:, :], SCRT[:, :], W)
                nc.sync.dma_start(orr[ci], O[:, :])
    nc.compile()
    return nc


def emit_chunk(nc, pp, Xt, Ut, Ot, SCRt, W):
    v = nc.vector
    a = nc.scalar
    scr = Scratch(SCRt, 62, W)

    xq = CompView(Xt, 13, 3, W)     # (qx,qy,qz,qw)
    xv = CompView(Xt, 13, 7, W)     # (vl0,vl1,vl2,va0,va1,va2)
    xpos = CompView(Xt, 13, 0, W)
    uu = CompView(Ut, 6, 0, W)
    ov = CompView(Ot, 13, 0, W)

    S1R = scr.alloc("S1R", 4)
    S2R = scr.alloc("S2R", 4)
    RHS1 = scr.alloc("RHS1", 6)
    RHS2 = scr.alloc("RHS2", 6)
    PL1 = scr.alloc("PL1", 3)
    PL2 = scr.alloc("PL2", 3)
    Q2 = scr.alloc("Q2", 4)
    V2 = scr.alloc("V2", 6)

    emit_dot(nc, pp, scr, xq, xv, uu, S1R, RHS1, PL1)

    # x2 = x + DT*k1 (q and v parts only feed f)
    v.scalar_tensor_tensor(Q2.ap(0), S1R.ap(0), -DT / 2, xq.ap(0), MULT, ADD)
    v.scalar_tensor_tensor(Q2.ap(1, 3), S1R.ap(1, 3), DT / 2, xq.ap(1, 3),
                           MULT, ADD)
    v.scalar_tensor_tensor(V2.ap(0, 3), RHS1.ap(0, 3), DT * pp["minv"][0],
                           xv.ap(0, 3), MULT, ADD)
    v.scalar_tensor_tensor(V2.ap(3, 3), RHS1.ap(3, 3), DT * pp["minv"][3],
                           xv.ap(3, 3), MULT, ADD)

    emit_dot(nc, pp, scr, PlanarView(Q2), PlanarView(V2), uu, S2R, RHS2, PL2)

    # ---- outputs ----
    TMP3 = scr.alloc("TMP3", 3)
    v.tensor_tensor(TMP3.ap(0, 3), PL1.ap(0, 3), PL2.ap(0, 3), ADD)
    v.scalar_tensor_tensor(ov.ap(0, 3), TMP3.ap(0, 3), DT / 2, xpos.ap(0, 3),
                           MULT, ADD)
    scr.free("TMP3", "PL1", "PL2")

    TMP6 = scr.alloc("TMP6", 6)
    v.tensor_tensor(TMP6.ap(0, 6), RHS1.ap(0, 6), RHS2.ap(0, 6), ADD)
    v.scalar_tensor_tensor(ov.ap(7, 3), TMP6.ap(0, 3),
                           DT / 2 * pp["minv"][0], xv.ap(0, 3), MULT, ADD)
    v.scalar_tensor_tensor(ov.ap(10, 3), TMP6.ap(3, 3),
                           DT / 2 * pp["minv"][3], xv.ap(3, 3), MULT, ADD)
    scr.free("TMP6", "RHS1", "RHS2")

    SS = scr.alloc("SS", 4)
    QR = scr.alloc("QR", 4)
    v.tensor_tensor(SS.ap(0, 4), S1R.ap(0, 4), S2R.ap(0, 4), ADD)
    v.scalar_tensor_tensor(QR.ap(0), SS.ap(0), -DT / 4, xq.ap(0), MULT, ADD)
    v.scalar_tensor_tensor(QR.ap(1, 3), SS.ap(1, 3), DT / 4, xq.ap(1, 3),
                           MULT, ADD)
    scr.free("SS", "S1R", "S2R")

    NQ = scr.alloc("NQ", 4)
    NS2 = scr.alloc("NS2", 2)
    NS1 = scr.alloc("NS1", 1)
    SQC = scr.alloc("SQC", 1)
    RINV = scr.alloc("RINV", 1)
    RTMP = scr.alloc("RTMP", 1)
    a.activation(NQ.ap(0, 4), QR.ap(0, 4), AF.Square)
    v.tensor_tensor(NS2.ap(0, 2), NQ.ap(0, 2), NQ.ap(2, 2), ADD)
    v.tensor_tensor(NS1.ap(0), NS2.ap(0), NS2.ap(1), ADD)
    a.activation(SQC.ap(0), NS1.ap(0), AF.Sqrt)
    v.reciprocal_approx_accurate(RINV.ap(0), SQC.ap(0), RTMP.ap(0))
    v.tensor_tensor(ov.ap(3, 4), QR.ap(0, 4), RINV.bcast(0, 4), MULT)
    scr.free("NQ", "NS2", "NS1", "SQC", "RINV", "RTMP", "QR", "Q2", "V2")


def emit_dot(nc, pp, scr, q, vv, uu, S, RHS, PL):
    """One f() evaluation.
    q: comps (0..3)=(qx,qy,qz,qw); vv: comps (0..5)=(vl,va); uu: comps 0..5.
    Outputs: S[4] = 2*pDot_ang with S[0] sign-flipped, RHS[6], PL[3]=pDot_lin.
    """
    v = nc.vector
    a = nc.scalar
    m1, ld, qd, c1, c2 = pp["m1"], pp["ld"], pp["qd"], pp["c1"], pp["c2"]

    # S rows (2*pDot_ang) first: needs the largest contiguous region (13)
    TP = scr.alloc("TP", 13)
    AS = scr.alloc("AS", 4)
    v.tensor_tensor(TP.ap(0, 3), q.ap(0, 3), vv.ap(3, 3), MULT)
    v.tensor_tensor(TP.ap(5, 3, -1), q.ap(1, 3), vv.ap(5, 3, -1), MULT)
    v.tensor_tensor(TP.ap(6, 2), q.ap(2, 2), vv.ap(3, 2), MULT)
    v.tensor_tensor(TP.ap(8), q.ap(0), vv.ap(5), MULT)
    v.tensor_tensor(TP.ap(9, 2), q.ap(0, 2, 3), vv.ap(4, 2), MULT)
    v.tensor_tensor(TP.ap(12), q.ap(1), vv.ap(3), MULT)
    v.tensor_tensor(AS.ap(0), TP.ap(0), TP.ap(1), ADD)
    v.tensor_tensor(AS.ap(1), TP.ap(3), TP.ap(5), ADD)
    v.tensor_tensor(AS.ap(2, 2), TP.ap(6, 2, 3), TP.ap(7, 2, 3), ADD)
    v.tensor_tensor(S.ap(0), AS.ap(0), TP.ap(2), ADD)
    v.tensor_tensor(S.ap(1, 3), AS.ap(1, 3), TP.ap(4, 3, 4), SUB)
    scr.free("TP", "AS")

    # quad products
    QD4 = scr.alloc("QD4", 4)   # [yy, zz, xx, yy]
    P1 = scr.alloc("P1", 3)     # [xy, xz, yz]
    P2 = scr.alloc("P2", 3)     # [zw, yw, xw]
    a.activation(QD4.ap(0, 2), q.ap(1, 2), AF.Square)              # yy, zz
    a.activation(QD4.ap(2), q.ap(0), AF.Square)                    # xx
    a.activation(QD4.ap(3), q.ap(1), AF.Square)                    # yy copy
    v.tensor_tensor(P1.ap(0, 2), q.bcast(0, 2), q.ap(1, 2), MULT)  # xy, xz
    v.tensor_tensor(P1.ap(2), q.ap(1), q.ap(2), MULT)              # yz
    v.tensor_tensor(P2.ap(0, 2), q.bcast(3, 2), q.ap(2, 2, -1), MULT)  # zw, yw
    v.tensor_tensor(P2.ap(2), q.ap(0), q.ap(3), MULT)              # xw

    QO = scr.alloc("QO", 6)     # [Qo10, Qo02, Qo21, Qo01, Qo20, Qo12]
    QDG = scr.alloc("QDG", 3)   # (yy+zz, zz+xx, xx+yy)
    v.tensor_tensor(QO.ap(0, 3), P1.ap(0, 3), P2.ap(0, 3), ADD)
    v.tensor_tensor(QO.ap(3, 3), P1.ap(0, 3), P2.ap(0, 3), SUB)
    v.tensor_tensor(QDG.ap(0, 3), QD4.ap(0, 3), QD4.ap(1, 3), ADD)
    scr.free("QD4", "P1", "P2")

    # pDot_lin = vl + 2*(Q @ vl)
    RD = scr.alloc("RD", 3)
    RO = scr.alloc("RO", 6)     # [R01, R02, R10, R12, R20, R21]
    T1 = scr.alloc("T1", 3)
    v.tensor_tensor(RD.ap(0, 3), QDG.ap(0, 3), vv.ap(0, 3), MULT)
    v.tensor_tensor(RO.ap(2, 2, -1), QO.ap(0, 2), vv.ap(0, 2, 2), MULT)
    v.tensor_tensor(RO.ap(0, 2, 4), QO.ap(3, 2), vv.ap(1, 2, -1), MULT)
    v.tensor_tensor(RO.ap(5), QO.ap(2), vv.ap(1), MULT)            # R21
    v.tensor_tensor(RO.ap(3), QO.ap(5), vv.ap(2), MULT)            # R12
    v.tensor_tensor(T1.ap(0, 3), RO.ap(0, 3, 2), RO.ap(1, 3, 2), ADD)
    v.tensor_tensor(T1.ap(0, 3), T1.ap(0, 3), RD.ap(0, 3), SUB)
    v.scalar_tensor_tensor(PL.ap(0, 3), T1.ap(0, 3), 2.0, vv.ap(0, 3),
                           MULT, ADD)
    scr.free("RD", "RO", "T1")

    # quadratic damping
    AB = scr.alloc("AB", 6)
    DVQ = scr.alloc("DVQ", 6)
    for i in range(6):
        a.activation(AB.ap(i), vv.ap(i), AF.Abs, scale=float(qd[i]))
    v.tensor_tensor(DVQ.ap(0, 6), AB.ap(0, 6), vv.ap(0, 6), MULT)
    scr.free("AB")

    # coriolis cross product va x vl
    PA = scr.alloc("PA", 3)
    PB = scr.alloc("PB", 3)
    CR = scr.alloc("CR", 3)
    v.tensor_tensor(PA.ap(0, 2), vv.ap(4, 2), vv.ap(2, 2, -2), MULT)
    v.tensor_tensor(PA.ap(2), vv.ap(3), vv.ap(1), MULT)
    v.tensor_tensor(PB.ap(1, 2), vv.ap(3, 2), vv.ap(2, 2, -2), MULT)
    v.tensor_tensor(PB.ap(0), vv.ap(5), vv.ap(1), MULT)
    v.tensor_tensor(CR.ap(0, 3), PA.ap(0, 3), PB.ap(0, 3), SUB)
    scr.free("PA", "PB")

    # rhs = u - Cv - Dv - g
    for i in range(6):
        v.scalar_tensor_tensor(RHS.ap(i), vv.ap(i), float(ld[i, i]), uu.ap(i),
                               MULT, ADD)
    for i in range(6):
        for j in range(6):
            if i != j and ld[i, j] != 0.0:
                v.scalar_tensor_tensor(RHS.ap(i), vv.ap(j), float(ld[i, j]),
                                       RHS.ap(i), MULT, ADD)
    neg = [i for i in range(6) if qd[i] < 0]
    pos = [i for i in range(6) if qd[i] >= 0 and qd[i] != 0.0]
    for grp, op in ((neg, SUB), (pos, ADD)):
        for i0, n in _runs(grp):
            v.tensor_tensor(RHS.ap(i0, n), RHS.ap(i0, n), DVQ.ap(i0 - 0, n)
                            if False else DVQ.ap(i0, n), op)
    scr.free("DVQ")
    v.scalar_tensor_tensor(RHS.ap(0, 3), CR.ap(0, 3), -m1, RHS.ap(0, 3),
                           MULT, ADD)
    scr.free("CR")

    # restoring: rot2 = (2*Qo20, 2*Qo21, 1-2*Qd2); rhs += [c1*rot2; c2 x rot2]
    TMPG = scr.alloc("TMPG", 1)
    v.scalar_tensor_tensor(RHS.ap(0, 2), QO.ap(4, 2, -2), 2.0 * c1,
                           RHS.ap(0, 2), MULT, ADD)
    v.tensor_scalar(TMPG.ap(0), QDG.ap(2), -2.0 * c1, c1, MULT, ADD)
    v.tensor_tensor(RHS.ap(2), RHS.ap(2), TMPG.ap(0), ADD)

    c2x, c2y, c2z = (float(c2[0]), float(c2[1]), float(c2[2]))
    gterms = [
        (3, [(QO, 2, -2.0 * c2z), (QDG, 2, -2.0 * c2y)], c2y),
        (4, [(QO, 4, 2.0 * c2z), (QDG, 2, 2.0 * c2x)], -c2x),
        (5, [(QO, 2, 2.0 * c2x), (QO, 4, -2.0 * c2y)], 0.0),
    ]
    for row, terms, const in gterms:
        terms = [(reg, s, co) for (reg, s, co) in terms if co != 0.0]
        if const != 0.0:
            if terms:
                reg, s, co = terms.pop(0)
                v.tensor_scalar(TMPG.ap(0), reg.ap(s), co, const, MULT, ADD)
                v.tensor_tensor(RHS.ap(row), RHS.ap(row), TMPG.ap(0), ADD)
            else:
                v.tensor_scalar(RHS.ap(row), RHS.ap(row), const, None, ADD)
        for reg, s, co in terms:
            v.scalar_tensor_tensor(RHS.ap(row), reg.ap(s), co, RHS.ap(row),
                                   MULT, ADD)
    scr.free("TMPG", "QO", "QDG")


def _runs(idxs):
    out = []
    for i in idxs:
        if out and i == out[-1][0] + out[-1][1]:
            out[-1] = (out[-1][0], out[-1][1] + 1)
        else:
            out.append((i, 1))
    return out


_CACHE = {}


def kernel(**inputs):
    from concourse.bass_utils import run_bass_kernel_spmd

    x = np.ascontiguousarray(np.asarray(inputs["x"], np.float32))
    u = np.ascontiguousarray(np.asarray(inputs["u"], np.float32))
    K = x.shape[0]
    assert K % N_CORES == 0
    K_core = K // N_CORES
    W = 512 if K_core % (P * 512) == 0 else 256
    assert K_core % (P * W) == 0

    pp = _extract_params(inputs)
    pp_key = (K_core, W, pp["m1"], pp["c1"], tuple(pp["minv"]),
              tuple(pp["qd"]), tuple(pp["c2"]), pp["ld"].tobytes())
    if pp_key not in _CACHE:
        _CACHE[pp_key] = build_program(pp, K_core, W)
    nc = _CACHE[pp_key]

    in_maps = []
    for k in range(N_CORES):
        sl = slice(k * K_core, (k + 1) * K_core)
        in_maps.append({"x": x[sl], "u": u[sl]})

    kwargs = dict(trace=True) if TRACE else {}
    res = run_bass_kernel_spmd(nc, in_maps, core_ids=list(range(N_CORES)),
                               **kwargs)
    LAST_RUN_INFO.clear()
    LAST_RUN_INFO.update(dict(
        exec_time_ns=res.exec_time_ns,
        mean_exec_time_ns=res.mean_exec_time_ns,
        profile_json=res.profile_json,
    ))
    out = np.empty((K, 13), np.float32)
    for k in range(N_CORES):
        out[k * K_core:(k + 1) * K_core] = res.results[k]["o"]
    return out


# revision 11
# speedup vs baseline: 1.4899x; 1.4899x over previous
"""Trainium2 Bass kernel for the AUV Fossen dynamics RK2 step (nn_AUVFossen).

Per row (batch K): x[13] = pos(3)+quat(x,y,z,w)(4)+v(6), u[6].
  k1 = f(x,u); k2 = f(x + DT*k1, u); out = normquat(x + DT/2*(k1+k2))

Sharding: pure data parallel over 8 NeuronCores (batch split).
Layout: batch-major. Each core holds its rows as SBUF tiles [128, C*W]
(W consecutive rows per partition, components interleaved w*C+c). All
compute is elementwise DVE/ACT ops on strided [128, W(,n)] APs; per-row
linear algebra is unrolled into grouped component ops.

For the structured parameters this problem ships:
  -Dv = linDamp@v + qd_i|v_i|v_i ; -Cv = [-m1*(va x vl); 0]
  -g  = [c1*rot2 ; c2 x rot2]  (rot2 = third row of rot)
  vDot = rhs / m_i  (mTot ~ diag(m1,m1,m1,m2,m2,m2))
"""

import os
import sys

for _p in ("/opt/trn_rl_repo", "/root/.axon_site/_ro/trn_rl_repo"):
    if os.path.isdir(_p) and _p not in sys.path:
        sys.path.insert(0, _p)

import numpy as np

import concourse.bacc as bacc
import concourse.bass as bass
import concourse.mybir as mybir
from concourse.alu_op_type import AluOpType
from concourse.tile import TileContext

F32 = mybir.dt.float32
AF = mybir.ActivationFunctionType
MULT = AluOpType.mult
ADD = AluOpType.add
SUB = AluOpType.subtract

DT = 0.1
GRAVITY = 9.81
DENSITY = 1028.0

N_CORES = 8
P = 128
NSLOTS = 91

TRACE = False          # set by test.py
LAST_RUN_INFO = {}


class CompView:
    """Component view of a [128, C*W] tile region with per-row interleave
    (addr = w*C + c).  ap(c0, n, cstep) -> AP of shape [128, n, W]
    (comp-outer, w-inner) so it can pair with planar scratch regions.
    The w-inner stride is C (non-unit): ops touching these views run at
    the DVE's slow ~2 cyc/elem addressing rate -- use sparingly."""

    def __init__(self, tile_ap, wstride, base, W):
        self.t = tile_ap.tensor
        self.off = tile_ap.offset + base
        self.part = tile_ap.ap[0]
        self.ws = wstride
        self.W = W

    def ap(self, c0, n=1, cstep=1):
        if n == 1:
            return bass.AP(self.t, self.off + c0, [self.part, [self.ws, self.W]])
        return bass.AP(self.t, self.off + c0,
                       [self.part, [cstep, n], [self.ws, self.W]])

    def bcast(self, c0, n):
        return bass.AP(self.t, self.off + c0,
                       [self.part, [0, n], [self.ws, self.W]])


class Region:
    """Contiguous run of planar scratch slots (slot = [128, W] plane).
    Grouped APs are (slot-outer, w-inner) => contiguous W-long innermost
    runs, which the DVE streams at full rate."""

    def __init__(self, scr, slot0, n):
        self.scr = scr
        self.slot0 = slot0
        self.n = n

    def ap(self, s0=0, n=1, sstep=1):
        scr = self.scr
        off = scr.off + (self.slot0 + s0) * scr.W
        if n == 1:
            return bass.AP(scr.t, off, [scr.part, [1, scr.W]])
        return bass.AP(scr.t, off,
                       [scr.part, [sstep * scr.W, n], [1, scr.W]])

    def bcast(self, s0, n):
        scr = self.scr
        off = scr.off + (self.slot0 + s0) * scr.W
        return bass.AP(scr.t, off, [scr.part, [0, n], [1, scr.W]])


class PlanarView:
    """CompView-compatible interface over a scratch Region (comp = slot)."""

    def __init__(self, reg):
        self.reg = reg

    def ap(self, c0, n=1, cstep=1):
        return self.reg.ap(c0, n, cstep)

    def bcast(self, c0, n):
        return self.reg.bcast(c0, n)


class Scratch:
    def __init__(self, tile_ap, nslots, W):
        self.t = tile_ap.tensor
        self.off = tile_ap.offset
        self.part = tile_ap.ap[0]
        self.W = W
        self.free_slots = set(range(nslots))
        self.regions = {}

    def alloc(self, name, n):
        fs = sorted(self.free_slots)
        run = None
        for i in range(len(fs) - n + 1):
            if fs[i + n - 1] - fs[i] == n - 1:
                run = fs[i]
                break
        assert run is not None, f"scratch OOM for {name}({n}); free={len(fs)}"
        for s in range(run, run + n):
            self.free_slots.remove(s)
        self.regions[name] = (run, n)
        return Region(self, run, n)

    def free(self, *names):
        for name in names:
            run, n = self.regions.pop(name)
            self.free_slots.update(range(run, run + n))


def _extract_params(inputs):
    mass = float(np.asarray(inputs["mass"]).reshape(-1)[0])
    volume = float(np.asarray(inputs["volume"]).reshape(-1)[0])
    cog = np.asarray(inputs["cog"], np.float64).reshape(3)
    cob = np.asarray(inputs["cob"], np.float64).reshape(3)
    mTot = np.asarray(inputs["mTot"], np.float64).reshape(6, 6)
    linDamp = np.asarray(inputs["linDamp"], np.float64).reshape(6, 6)
    linDampFow = np.asarray(inputs["linDampFow"], np.float64).reshape(6, 6)
    quadDamp = np.asarray(inputs["quadDamp"], np.float64).reshape(6, 6)

    scale = max(np.abs(mTot).max(), 1e-30)
    tl, tr = mTot[0:3, 0:3], mTot[0:3, 3:6]
    bl, br = mTot[3:6, 0:3], mTot[3:6, 3:6]
    m1 = float(np.trace(tl) / 3.0)
    m2 = float(np.trace(br) / 3.0)
    structured = (
        np.abs(tl - m1 * np.eye(3)).max() < 1e-5 * scale
        and np.abs(br - m2 * np.eye(3)).max() < 1e-5 * scale
        and np.abs(tr).max() < 1e-5 * scale
        and np.abs(bl).max() < 1e-5 * scale
    )
    if not structured:
        raise NotImplementedError("unstructured mTot not supported")
    if np.abs(linDampFow).max() > 1e-30:
        raise NotImplementedError("nonzero linDampFow not supported")

    minv = np.diag(np.linalg.inv(mTot))
    ld = linDamp.copy()                 # -Dv_lin = +linDamp @ v
    qd = np.diag(quadDamp)              # only diag of quadDamp matters
    c1 = GRAVITY * (volume * DENSITY - mass)
    c2 = -mass * GRAVITY * cog + volume * DENSITY * GRAVITY * cob
    return dict(m1=m1, minv=minv, ld=ld, qd=qd, c1=float(c1), c2=c2)


def build_program(pp, K_core, W):
    assert K_core % (P * W) == 0
    n_chunks = K_core // (P * W)

    nc = bacc.Bacc("TRN2", target_bir_lowering=False, debug=False,
                   num_devices=N_CORES)
    x_d = nc.dram_tensor("x", (K_core, 13), F32, kind="ExternalInput")
    u_d = nc.dram_tensor("u", (K_core, 6), F32, kind="ExternalInput")
    o_d = nc.dram_tensor("o", (K_core, 13), F32, kind="ExternalOutput")

    xr = x_d.rearrange("(n p w) c -> n p (w c)", p=P, w=W)
    ur = u_d.rearrange("(n p w) c -> n p (w c)", p=P, w=W)
    orr = o_d.rearrange("(n p w) c -> n p (w c)", p=P, w=W)

    with TileContext(nc) as tc:
        with tc.tile_pool(name="io", bufs=2 if n_chunks > 1 else 1) as iop, \
             tc.tile_pool(name="scr", bufs=1) as scrp:
            for ci in range(n_chunks):
                X = iop.tile([P, 13 * W], F32, tag="X")
                U = iop.tile([P, 6 * W], F32, tag="U")
                O = iop.tile([P, 13 * W], F32, tag="O")
                SCRT = scrp.tile([P, NSLOTS * W], F32, tag="SCR")
                nc.sync.dma_start(X[:, :], xr[ci])
                nc.sync.dma_start(U[:, :], ur[ci])
                emit_chunk(nc, pp, X[:, :], U[:, :], O[:, :], SCRT[:, :], W)
                nc.sync.dma_start(orr[ci], O[:, :])
    nc.compile()
    return nc


def emit_chunk(nc, pp, Xt, Ut, Ot, SCRt, W):
    v = nc.vector
    a = nc.scalar
    scr = Scratch(SCRt, NSLOTS, W)

    xall = CompView(Xt, 13, 0, W)
    xpos = CompView(Xt, 13, 0, W)
    oint = CompView(Ot, 13, 0, W)
    uint = CompView(Ut, 6, 0, W)

    # planarize inputs on the (otherwise idle) scalar engine so every DVE
    # op downstream streams contiguous W-long runs at full rate
    XPL = scr.alloc("XPL", 10)      # (qx,qy,qz,qw, vl0..2, va0..2)
    UPL = scr.alloc("UPL", 6)
    OPL = scr.alloc("OPL", 13)
    a.activation(XPL.ap(0, 10), xall.ap(3, 10), AF.Copy)
    a.activation(UPL.ap(0, 6), uint.ap(0, 6), AF.Copy)

    xq = PlanarView(Region(scr, XPL.slot0, 4))
    xv = PlanarView(Region(scr, XPL.slot0 + 4, 6))
    uu = PlanarView(UPL)
    ov = PlanarView(OPL)

    S1R = scr.alloc("S1R", 4)
    S2R = scr.alloc("S2R", 4)
    RHS1 = scr.alloc("RHS1", 6)
    RHS2 = scr.alloc("RHS2", 6)
    PL1 = scr.alloc("PL1", 3)
    PL2 = scr.alloc("PL2", 3)
    Q2 = scr.alloc("Q2", 4)
    V2 = scr.alloc("V2", 6)

    emit_dot(nc, pp, scr, xq, xv, uu, S1R, RHS1, PL1)

    # x2 = x + DT*k1 (q and v parts only feed f)
    v.scalar_tensor_tensor(Q2.ap(0), S1R.ap(0), -DT / 2, xq.ap(0), MULT, ADD)
    v.scalar_tensor_tensor(Q2.ap(1, 3), S1R.ap(1, 3), DT / 2, xq.ap(1, 3),
                           MULT, ADD)
    v.scalar_tensor_tensor(V2.ap(0, 3), RHS1.ap(0, 3), DT * pp["minv"][0],
                           xv.ap(0, 3), MULT, ADD)
    v.scalar_tensor_tensor(V2.ap(3, 3), RHS1.ap(3, 3), DT * pp["minv"][3],
                           xv.ap(3, 3), MULT, ADD)

    emit_dot(nc, pp, scr, PlanarView(Q2), PlanarView(V2), uu, S2R, RHS2, PL2)

    # ---- outputs ----
    TMP3 = scr.alloc("TMP3", 3)
    v.tensor_tensor(TMP3.ap(0, 3), PL1.ap(0, 3), PL2.ap(0, 3), ADD)
    v.scalar_tensor_tensor(ov.ap(0, 3), TMP3.ap(0, 3), DT / 2, xpos.ap(0, 3),
                           MULT, ADD)
    scr.free("TMP3", "PL1", "PL2")

    TMP6 = scr.alloc("TMP6", 6)
    v.tensor_tensor(TMP6.ap(0, 6), RHS1.ap(0, 6), RHS2.ap(0, 6), ADD)
    v.scalar_tensor_tensor(ov.ap(7, 3), TMP6.ap(0, 3),
                           DT / 2 * pp["minv"][0], xv.ap(0, 3), MULT, ADD)
    v.scalar_tensor_tensor(ov.ap(10, 3), TMP6.ap(3, 3),
                           DT / 2 * pp["minv"][3], xv.ap(3, 3), MULT, ADD)
    scr.free("TMP6", "RHS1", "RHS2")

    SS = scr.alloc("SS", 4)
    QR = scr.alloc("QR", 4)
    v.tensor_tensor(SS.ap(0, 4), S1R.ap(0, 4), S2R.ap(0, 4), ADD)
    v.scalar_tensor_tensor(QR.ap(0), SS.ap(0), -DT / 4, xq.ap(0), MULT, ADD)
    v.scalar_tensor_tensor(QR.ap(1, 3), SS.ap(1, 3), DT / 4, xq.ap(1, 3),
                           MULT, ADD)
    scr.free("SS", "S1R", "S2R")

    NQ = scr.alloc("NQ", 4)
    NS2 = scr.alloc("NS2", 2)
    NS1 = scr.alloc("NS1", 1)
    SQC = scr.alloc("SQC", 1)
    RINV = scr.alloc("RINV", 1)
    RTMP = scr.alloc("RTMP", 1)
    a.activation(NQ.ap(0, 4), QR.ap(0, 4), AF.Square)
    v.tensor_tensor(NS2.ap(0, 2), NQ.ap(0, 2), NQ.ap(2, 2), ADD)
    v.tensor_tensor(NS1.ap(0), NS2.ap(0), NS2.ap(1), ADD)
    a.activation(SQC.ap(0), NS1.ap(0), AF.Sqrt)
    v.reciprocal_approx_accurate(RINV.ap(0), SQC.ap(0), RTMP.ap(0))
    v.tensor_tensor(ov.ap(3, 4), QR.ap(0, 4), RINV.bcast(0, 4), MULT)
    scr.free("NQ", "NS2", "NS1", "SQC", "RINV", "RTMP", "QR", "Q2", "V2")

    # de-planarize the assembled output back to row-interleaved layout
    a.activation(oint.ap(0, 13), OPL.ap(0, 13), AF.Copy)
    scr.free("XPL", "UPL", "OPL")


def emit_dot(nc, pp, scr, q, vv, uu, S, RHS, PL):
    """One f() evaluation.
    q: comps (0..3)=(qx,qy,qz,qw); vv: comps (0..5)=(vl,va); uu: comps 0..5.
    Outputs: S[4] = 2*pDot_ang with S[0] sign-flipped, RHS[6], PL[3]=pDot_lin.
    """
    v = nc.vector
    a = nc.scalar
    m1, ld, qd, c1, c2 = pp["m1"], pp["ld"], pp["qd"], pp["c1"], pp["c2"]

    # S rows (2*pDot_ang) first: needs the largest contiguous region (13)
    TP = scr.alloc("TP", 13)
    AS = scr.alloc("AS", 4)
    v.tensor_tensor(TP.ap(0, 3), q.ap(0, 3), vv.ap(3, 3), MULT)
    v.tensor_tensor(TP.ap(5, 3, -1), q.ap(1, 3), vv.ap(5, 3, -1), MULT)
    v.tensor_tensor(TP.ap(6, 2), q.ap(2, 2), vv.ap(3, 2), MULT)
    v.tensor_tensor(TP.ap(8), q.ap(0), vv.ap(5), MULT)
    v.tensor_tensor(TP.ap(9, 2), q.ap(0, 2, 3), vv.ap(4, 2), MULT)
    v.tensor_tensor(TP.ap(12), q.ap(1), vv.ap(3), MULT)
    v.tensor_tensor(AS.ap(0), TP.ap(0), TP.ap(1), ADD)
    v.tensor_tensor(AS.ap(1), TP.ap(3), TP.ap(5), ADD)
    v.tensor_tensor(AS.ap(2, 2), TP.ap(6, 2, 3), TP.ap(7, 2, 3), ADD)
    v.tensor_tensor(S.ap(0), AS.ap(0), TP.ap(2), ADD)
    v.tensor_tensor(S.ap(1, 3), AS.ap(1, 3), TP.ap(4, 3, 4), SUB)
    scr.free("TP", "AS")

    # quad products
    QD4 = scr.alloc("QD4", 4)   # [yy, zz, xx, yy]
    P1 = scr.alloc("P1", 3)     # [xy, xz, yz]
    P2 = scr.alloc("P2", 3)     # [zw, yw, xw]
    a.activation(QD4.ap(0, 2), q.ap(1, 2), AF.Square)              # yy, zz
    a.activation(QD4.ap(2), q.ap(0), AF.Square)                    # xx
    a.activation(QD4.ap(3), q.ap(1), AF.Square)                    # yy copy
    v.tensor_tensor(P1.ap(0, 2), q.bcast(0, 2), q.ap(1, 2), MULT)  # xy, xz
    v.tensor_tensor(P1.ap(2), q.ap(1), q.ap(2), MULT)              # yz
    v.tensor_tensor(P2.ap(0, 2), q.bcast(3, 2), q.ap(2, 2, -1), MULT)  # zw, yw
    v.tensor_tensor(P2.ap(2), q.ap(0), q.ap(3), MULT)              # xw

    QO = scr.alloc("QO", 6)     # [Qo10, Qo02, Qo21, Qo01, Qo20, Qo12]
    QDG = scr.alloc("QDG", 3)   # (yy+zz, zz+xx, xx+yy)
    v.tensor_tensor(QO.ap(0, 3), P1.ap(0, 3), P2.ap(0, 3), ADD)
    v.tensor_tensor(QO.ap(3, 3), P1.ap(0, 3), P2.ap(0, 3), SUB)
    v.tensor_tensor(QDG.ap(0, 3), QD4.ap(0, 3), QD4.ap(1, 3), ADD)
    scr.free("QD4", "P1", "P2")

    # pDot_lin = vl + 2*(Q @ vl)
    RD = scr.alloc("RD", 3)
    RO = scr.alloc("RO", 6)     # [R01, R02, R10, R12, R20, R21]
    T1 = scr.alloc("T1", 3)
    v.tensor_tensor(RD.ap(0, 3), QDG.ap(0, 3), vv.ap(0, 3), MULT)
    v.tensor_tensor(RO.ap(2, 2, -1), QO.ap(0, 2), vv.ap(0, 2, 2), MULT)
    v.tensor_tensor(RO.ap(0, 2, 4), QO.ap(3, 2), vv.ap(1, 2, -1), MULT)
    v.tensor_tensor(RO.ap(5), QO.ap(2), vv.ap(1), MULT)            # R21
    v.tensor_tensor(RO.ap(3), QO.ap(5), vv.ap(2), MULT)            # R12
    v.tensor_tensor(T1.ap(0, 3), RO.ap(0, 3, 2), RO.ap(1, 3, 2), ADD)
    v.tensor_tensor(T1.ap(0, 3), T1.ap(0, 3), RD.ap(0, 3), SUB)
    v.scalar_tensor_tensor(PL.ap(0, 3), T1.ap(0, 3), 2.0, vv.ap(0, 3),
                           MULT, ADD)
    scr.free("RD", "RO", "T1")

    # quadratic damping
    AB = scr.alloc("AB", 6)
    DVQ = scr.alloc("DVQ", 6)
    for i in range(6):
        a.activation(AB.ap(i), vv.ap(i), AF.Abs, scale=float(qd[i]))
    v.tensor_tensor(DVQ.ap(0, 6), AB.ap(0, 6), vv.ap(0, 6), MULT)
    scr.free("AB")

    # coriolis cross product va x vl
    PA = scr.alloc("PA", 3)
    PB = scr.alloc("PB", 3)
    CR = scr.alloc("CR", 3)
    v.tensor_tensor(PA.ap(0, 2), vv.ap(4, 2), vv.ap(2, 2, -2), MULT)
    v.tensor_tensor(PA.ap(2), vv.ap(3), vv.ap(1), MULT)
    v.tensor_tensor(PB.ap(1, 2), vv.ap(3, 2), vv.ap(2, 2, -2), MULT)
    v.tensor_tensor(PB.ap(0), vv.ap(5), vv.ap(1), MULT)
    v.tensor_tensor(CR.ap(0, 3), PA.ap(0, 3), PB.ap(0, 3), SUB)
    scr.free("PA", "PB")

    # rhs = u - Cv - Dv - g
    for i in range(6):
        v.scalar_tensor_tensor(RHS.ap(i), vv.ap(i), float(ld[i, i]), uu.ap(i),
                               MULT, ADD)
    for i in range(6):
        for j in range(6):
            if i != j and ld[i, j] != 0.0:
                v.scalar_tensor_tensor(RHS.ap(i), vv.ap(j), float(ld[i, j]),
                                       RHS.ap(i), MULT, ADD)
    neg = [i for i in range(6) if qd[i] < 0]
    pos = [i for i in range(6) if qd[i] >= 0 and qd[i] != 0.0]
    for grp, op in ((neg, SUB), (pos, ADD)):
        for i0, n in _runs(grp):
            v.tensor_tensor(RHS.ap(i0, n), RHS.ap(i0, n), DVQ.ap(i0 - 0, n)
                            if False else DVQ.ap(i0, n), op)
    scr.free("DVQ")
    v.scalar_tensor_tensor(RHS.ap(0, 3), CR.ap(0, 3), -m1, RHS.ap(0, 3),
                           MULT, ADD)
    scr.free("CR")

    # restoring: rot2 = (2*Qo20, 2*Qo21, 1-2*Qd2); rhs += [c1*rot2; c2 x rot2]
    TMPG = scr.alloc("TMPG", 1)
    v.scalar_tensor_tensor(RHS.ap(0, 2), QO.ap(4, 2, -2), 2.0 * c1,
                           RHS.ap(0, 2), MULT, ADD)
    v.tensor_scalar(TMPG.ap(0), QDG.ap(2), -2.0 * c1, c1, MULT, ADD)
    v.tensor_tensor(RHS.ap(2), RHS.ap(2), TMPG.ap(0), ADD)

    c2x, c2y, c2z = (float(c2[0]), float(c2[1]), float(c2[2]))
    gterms = [
        (3, [(QO, 2, -2.0 * c2z), (QDG, 2, -2.0 * c2y)], c2y),
        (4, [(QO, 4, 2.0 * c2z), (QDG, 2, 2.0 * c2x)], -c2x),
        (5, [(QO, 2, 2.0 * c2x), (QO, 4, -2.0 * c2y)], 0.0),
    ]
    for row, terms, const in gterms:
        terms = [(reg, s, co) for (reg, s, co) in terms if co != 0.0]
        if const != 0.0:
            if terms:
                reg, s, co = terms.pop(0)
                v.tensor_scalar(TMPG.ap(0), reg.ap(s), co, const, MULT, ADD)
                v.tensor_tensor(RHS.ap(row), RHS.ap(row), TMPG.ap(0), ADD)
            else:
                v.tensor_scalar(RHS.ap(row), RHS.ap(row), const, None, ADD)
        for reg, s, co in terms:
            v.scalar_tensor_tensor(RHS.ap(row), reg.ap(s), co, RHS.ap(row),
                                   MULT, ADD)
    scr.free("TMPG", "QO", "QDG")


def _runs(idxs):
    out = []
    for i in idxs:
        if out and i == out[-1][0] + out[-1][1]:
            out[-1] = (out[-1][0], out[-1][1] + 1)
        else:
            out.append((i, 1))
    return out


_CACHE = {}


def kernel(**inputs):
    from concourse.bass_utils import run_bass_kernel_spmd

    x = np.ascontiguousarray(np.asarray(inputs["x"], np.float32))
    u = np.ascontiguousarray(np.asarray(inputs["u"], np.float32))
    K = x.shape[0]
    assert K % N_CORES == 0
    K_core = K // N_CORES
    W = 256
    assert K_core % (P * W) == 0

    pp = _extract_params(inputs)
    pp_key = (K_core, W, pp["m1"], pp["c1"], tuple(pp["minv"]),
              tuple(pp["qd"]), tuple(pp["c2"]), pp["ld"].tobytes())
    if pp_key not in _CACHE:
        _CACHE[pp_key] = build_program(pp, K_core, W)
    nc = _CACHE[pp_key]

    in_maps = []
    for k in range(N_CORES):
        sl = slice(k * K_core, (k + 1) * K_core)
        in_maps.append({"x": x[sl], "u": u[sl]})

    kwargs = dict(trace=True) if TRACE else {}
    res = run_bass_kernel_spmd(nc, in_maps, core_ids=list(range(N_CORES)),
                               **kwargs)
    LAST_RUN_INFO.clear()
    LAST_RUN_INFO.update(dict(
        exec_time_ns=res.exec_time_ns,
        mean_exec_time_ns=res.mean_exec_time_ns,
        profile_json=res.profile_json,
    ))
    out = np.empty((K, 13), np.float32)
    for k in range(N_CORES):
        out[k * K_core:(k + 1) * K_core] = res.results[k]["o"]
    return out


# revision 14
# speedup vs baseline: 1.6007x; 1.0744x over previous
"""Trainium2 Bass kernel for the AUV Fossen dynamics RK2 step (nn_AUVFossen).

Per row (batch K): x[13] = pos(3)+quat(x,y,z,w)(4)+v(6), u[6].
  k1 = f(x,u); k2 = f(x + DT*k1, u); out = normquat(x + DT/2*(k1+k2))

Sharding: pure data parallel over 8 NeuronCores (batch split).
Layout: batch-major. Each core holds its rows as SBUF tiles [128, C*W]
(W consecutive rows per partition, components interleaved w*C+c). All
compute is elementwise DVE/ACT ops on strided [128, W(,n)] APs; per-row
linear algebra is unrolled into grouped component ops.

For the structured parameters this problem ships:
  -Dv = linDamp@v + qd_i|v_i|v_i ; -Cv = [-m1*(va x vl); 0]
  -g  = [c1*rot2 ; c2 x rot2]  (rot2 = third row of rot)
  vDot = rhs / m_i  (mTot ~ diag(m1,m1,m1,m2,m2,m2))
"""

import os
import sys

for _p in ("/opt/trn_rl_repo", "/root/.axon_site/_ro/trn_rl_repo"):
    if os.path.isdir(_p) and _p not in sys.path:
        sys.path.insert(0, _p)

import numpy as np

import concourse.bacc as bacc
import concourse.bass as bass
import concourse.mybir as mybir
from concourse.alu_op_type import AluOpType
from concourse.tile import TileContext

F32 = mybir.dt.float32
AF = mybir.ActivationFunctionType
MULT = AluOpType.mult
ADD = AluOpType.add
SUB = AluOpType.subtract

DT = 0.1
GRAVITY = 9.81
DENSITY = 1028.0

N_CORES = 8
P = 128
NSLOTS = 91

TRACE = False          # set by test.py
LAST_RUN_INFO = {}


class CompView:
    """Component view of a [128, C*W] tile region with per-row interleave
    (addr = w*C + c).  ap(c0, n, cstep) -> AP of shape [128, n, W]
    (comp-outer, w-inner) so it can pair with planar scratch regions.
    The w-inner stride is C (non-unit): ops touching these views run at
    the DVE's slow ~2 cyc/elem addressing rate -- use sparingly."""

    def __init__(self, tile_ap, wstride, base, W):
        self.t = tile_ap.tensor
        self.off = tile_ap.offset + base
        self.part = tile_ap.ap[0]
        self.ws = wstride
        self.W = W

    def ap(self, c0, n=1, cstep=1):
        if n == 1:
            return bass.AP(self.t, self.off + c0, [self.part, [self.ws, self.W]])
        return bass.AP(self.t, self.off + c0,
                       [self.part, [cstep, n], [self.ws, self.W]])

    def bcast(self, c0, n):
        return bass.AP(self.t, self.off + c0,
                       [self.part, [0, n], [self.ws, self.W]])


class Region:
    """Contiguous run of planar scratch slots (slot = [128, W] plane).
    Grouped APs are (slot-outer, w-inner) => contiguous W-long innermost
    runs, which the DVE streams at full rate."""

    def __init__(self, scr, slot0, n):
        self.scr = scr
        self.slot0 = slot0
        self.n = n

    def ap(self, s0=0, n=1, sstep=1):
        scr = self.scr
        off = scr.off + (self.slot0 + s0) * scr.W
        if n == 1:
            return bass.AP(scr.t, off, [scr.part, [1, scr.W]])
        return bass.AP(scr.t, off,
                       [scr.part, [sstep * scr.W, n], [1, scr.W]])

    def bcast(self, s0, n):
        scr = self.scr
        off = scr.off + (self.slot0 + s0) * scr.W
        return bass.AP(scr.t, off, [scr.part, [0, n], [1, scr.W]])


class PlanarView:
    """CompView-compatible interface over a scratch Region (comp = slot)."""

    def __init__(self, reg):
        self.reg = reg

    def ap(self, c0, n=1, cstep=1):
        return self.reg.ap(c0, n, cstep)

    def bcast(self, c0, n):
        return self.reg.bcast(c0, n)


class Scratch:
    def __init__(self, tile_ap, nslots, W):
        self.t = tile_ap.tensor
        self.off = tile_ap.offset
        self.part = tile_ap.ap[0]
        self.W = W
        self.free_slots = set(range(nslots))
        self.regions = {}

    def alloc(self, name, n):
        fs = sorted(self.free_slots)
        run = None
        for i in range(len(fs) - n + 1):
            if fs[i + n - 1] - fs[i] == n - 1:
                run = fs[i]
                break
        assert run is not None, f"scratch OOM for {name}({n}); free={len(fs)}"
        for s in range(run, run + n):
            self.free_slots.remove(s)
        self.regions[name] = (run, n)
        return Region(self, run, n)

    def free(self, *names):
        for name in names:
            run, n = self.regions.pop(name)
            self.free_slots.update(range(run, run + n))


def _extract_params(inputs):
    mass = float(np.asarray(inputs["mass"]).reshape(-1)[0])
    volume = float(np.asarray(inputs["volume"]).reshape(-1)[0])
    cog = np.asarray(inputs["cog"], np.float64).reshape(3)
    cob = np.asarray(inputs["cob"], np.float64).reshape(3)
    mTot = np.asarray(inputs["mTot"], np.float64).reshape(6, 6)
    linDamp = np.asarray(inputs["linDamp"], np.float64).reshape(6, 6)
    linDampFow = np.asarray(inputs["linDampFow"], np.float64).reshape(6, 6)
    quadDamp = np.asarray(inputs["quadDamp"], np.float64).reshape(6, 6)

    scale = max(np.abs(mTot).max(), 1e-30)
    tl, tr = mTot[0:3, 0:3], mTot[0:3, 3:6]
    bl, br = mTot[3:6, 0:3], mTot[3:6, 3:6]
    m1 = float(np.trace(tl) / 3.0)
    m2 = float(np.trace(br) / 3.0)
    structured = (
        np.abs(tl - m1 * np.eye(3)).max() < 1e-5 * scale
        and np.abs(br - m2 * np.eye(3)).max() < 1e-5 * scale
        and np.abs(tr).max() < 1e-5 * scale
        and np.abs(bl).max() < 1e-5 * scale
    )
    if not structured:
        raise NotImplementedError("unstructured mTot not supported")
    if np.abs(linDampFow).max() > 1e-30:
        raise NotImplementedError("nonzero linDampFow not supported")

    minv = np.diag(np.linalg.inv(mTot))
    ld = linDamp.copy()                 # -Dv_lin = +linDamp @ v
    qd = np.diag(quadDamp)              # only diag of quadDamp matters
    c1 = GRAVITY * (volume * DENSITY - mass)
    c2 = -mass * GRAVITY * cog + volume * DENSITY * GRAVITY * cob
    return dict(m1=m1, minv=minv, ld=ld, qd=qd, c1=float(c1), c2=c2)


def build_program(pp, K_core, W):
    assert K_core % (P * W) == 0
    n_chunks = K_core // (P * W)

    nc = bacc.Bacc("TRN2", target_bir_lowering=False, debug=False,
                   num_devices=N_CORES)
    x_d = nc.dram_tensor("x", (K_core, 13), F32, kind="ExternalInput")
    u_d = nc.dram_tensor("u", (K_core, 6), F32, kind="ExternalInput")
    o_d = nc.dram_tensor("o", (K_core, 13), F32, kind="ExternalOutput")

    xr = x_d.rearrange("(n p w) c -> n p (w c)", p=P, w=W)
    ur = u_d.rearrange("(n p w) c -> n p (w c)", p=P, w=W)
    orr = o_d.rearrange("(n p w) c -> n p (w c)", p=P, w=W)

    nb = 2 if n_chunks > 1 else 1
    with TileContext(nc) as tc:
        with tc.tile_pool(name="io", bufs=nb) as iop, \
             tc.tile_pool(name="pv", bufs=nb) as pvp, \
             tc.tile_pool(name="scr", bufs=1) as scrp:
            for ci in range(n_chunks):
                X = iop.tile([P, 13 * W], F32, tag="X")
                U = iop.tile([P, 6 * W], F32, tag="U")
                O = iop.tile([P, 13 * W], F32, tag="O")
                XPLT = pvp.tile([P, 10 * W], F32, tag="XPL")
                UPLT = pvp.tile([P, 6 * W], F32, tag="UPL")
                SCRT = scrp.tile([P, NSLOTS * W], F32, tag="SCR")
                nc.sync.dma_start(X[:, :], xr[ci])
                nc.sync.dma_start(U[:, :], ur[ci])
                emit_chunk(nc, pp, X[:, :], U[:, :], O[:, :],
                           XPLT[:, :], UPLT[:, :], SCRT[:, :], W)
                nc.sync.dma_start(orr[ci], O[:, :])
    nc.compile()
    return nc


def emit_chunk(nc, pp, Xt, Ut, Ot, XPLt, UPLt, SCRt, W):
    v = nc.vector
    a = nc.scalar
    scr = Scratch(SCRt, NSLOTS, W)
    xscr = Scratch(XPLt, 10, W)
    uscr = Scratch(UPLt, 6, W)

    xall = CompView(Xt, 13, 0, W)
    xpos = CompView(Xt, 13, 0, W)
    oint = CompView(Ot, 13, 0, W)
    uint = CompView(Ut, 6, 0, W)

    # planarize inputs on the (otherwise idle) scalar engine so every DVE
    # op downstream streams contiguous W-long runs at full rate; q+va
    # first so the DVE's opening t-product block can start early
    XPL = xscr.alloc("XPL", 10)     # (qx,qy,qz,qw, vl0..2, va0..2)
    UPL = uscr.alloc("UPL", 6)
    OPL = scr.alloc("OPL", 13)
    a.activation(XPL.ap(0, 4), xall.ap(3, 4), AF.Copy)    # q
    a.activation(XPL.ap(7, 3), xall.ap(10, 3), AF.Copy)   # va
    a.activation(XPL.ap(4, 3), xall.ap(7, 3), AF.Copy)    # vl
    a.activation(UPL.ap(0, 6), uint.ap(0, 6), AF.Copy)

    xq = PlanarView(Region(xscr, 0, 4))
    xv = PlanarView(Region(xscr, 4, 6))
    uu = PlanarView(UPL)
    ov = PlanarView(OPL)

    S1R = scr.alloc("S1R", 4)
    S2R = scr.alloc("S2R", 4)
    RHS1 = scr.alloc("RHS1", 6)
    RHS2 = scr.alloc("RHS2", 6)
    PL1 = scr.alloc("PL1", 3)
    PL2 = scr.alloc("PL2", 3)
    Q2 = scr.alloc("Q2", 4)
    V2 = scr.alloc("V2", 6)

    emit_dot(nc, pp, scr, xq, xv, uu, S1R, RHS1, PL1)

    # x2 = x + DT*k1 (q and v parts only feed f)
    v.scalar_tensor_tensor(Q2.ap(0), S1R.ap(0), -DT / 2, xq.ap(0), MULT, ADD)
    v.scalar_tensor_tensor(Q2.ap(1, 3), S1R.ap(1, 3), DT / 2, xq.ap(1, 3),
                           MULT, ADD)
    v.scalar_tensor_tensor(V2.ap(0, 3), RHS1.ap(0, 3), DT * pp["minv"][0],
                           xv.ap(0, 3), MULT, ADD)
    v.scalar_tensor_tensor(V2.ap(3, 3), RHS1.ap(3, 3), DT * pp["minv"][3],
                           xv.ap(3, 3), MULT, ADD)

    emit_dot(nc, pp, scr, PlanarView(Q2), PlanarView(V2), uu, S2R, RHS2, PL2)

    # ---- outputs ----
    TMP3 = scr.alloc("TMP3", 3)
    v.tensor_tensor(TMP3.ap(0, 3), PL1.ap(0, 3), PL2.ap(0, 3), ADD)
    v.scalar_tensor_tensor(ov.ap(0, 3), TMP3.ap(0, 3), DT / 2, xpos.ap(0, 3),
                           MULT, ADD)
    a.activation(oint.ap(0, 3), OPL.ap(0, 3), AF.Copy)
    scr.free("TMP3", "PL1", "PL2")

    TMP6 = scr.alloc("TMP6", 6)
    v.tensor_tensor(TMP6.ap(0, 6), RHS1.ap(0, 6), RHS2.ap(0, 6), ADD)
    v.scalar_tensor_tensor(ov.ap(7, 3), TMP6.ap(0, 3),
                           DT / 2 * pp["minv"][0], xv.ap(0, 3), MULT, ADD)
    v.scalar_tensor_tensor(ov.ap(10, 3), TMP6.ap(3, 3),
                           DT / 2 * pp["minv"][3], xv.ap(3, 3), MULT, ADD)
    a.activation(oint.ap(7, 6), OPL.ap(7, 6), AF.Copy)
    scr.free("TMP6", "RHS1", "RHS2")

    SS = scr.alloc("SS", 4)
    QR = scr.alloc("QR", 4)
    v.tensor_tensor(SS.ap(0, 4), S1R.ap(0, 4), S2R.ap(0, 4), ADD)
    v.scalar_tensor_tensor(QR.ap(0), SS.ap(0), -DT / 4, xq.ap(0), MULT, ADD)
    v.scalar_tensor_tensor(QR.ap(1, 3), SS.ap(1, 3), DT / 4, xq.ap(1, 3),
                           MULT, ADD)
    scr.free("SS", "S1R", "S2R")

    NQ = scr.alloc("NQ", 4)
    NS2 = scr.alloc("NS2", 2)
    NS1 = scr.alloc("NS1", 1)
    SQC = scr.alloc("SQC", 1)
    RINV = scr.alloc("RINV", 1)
    RTMP = scr.alloc("RTMP", 1)
    a.activation(NQ.ap(0, 4), QR.ap(0, 4), AF.Square)
    v.tensor_tensor(NS2.ap(0, 2), NQ.ap(0, 2), NQ.ap(2, 2), ADD)
    v.tensor_tensor(NS1.ap(0), NS2.ap(0), NS2.ap(1), ADD)
    a.activation(SQC.ap(0), NS1.ap(0), AF.Sqrt)
    v.reciprocal_approx_accurate(RINV.ap(0), SQC.ap(0), RTMP.ap(0))
    v.tensor_tensor(ov.ap(3, 4), QR.ap(0, 4), RINV.bcast(0, 4), MULT)
    a.activation(oint.ap(3, 4), OPL.ap(3, 4), AF.Copy)
    scr.free("NQ", "NS2", "NS1", "SQC", "RINV", "RTMP", "QR", "Q2", "V2", "OPL")


def emit_dot(nc, pp, scr, q, vv, uu, S, RHS, PL):
    """One f() evaluation.
    q: comps (0..3)=(qx,qy,qz,qw); vv: comps (0..5)=(vl,va); uu: comps 0..5.
    Outputs: S[4] = 2*pDot_ang with S[0] sign-flipped, RHS[6], PL[3]=pDot_lin.
    """
    v = nc.vector
    a = nc.scalar
    m1, ld, qd, c1, c2 = pp["m1"], pp["ld"], pp["qd"], pp["c1"], pp["c2"]

    # S rows (2*pDot_ang) first: needs the largest contiguous region (13)
    TP = scr.alloc("TP", 13)
    AS = scr.alloc("AS", 4)
    v.tensor_tensor(TP.ap(0, 3), q.ap(0, 3), vv.ap(3, 3), MULT)
    v.tensor_tensor(TP.ap(5, 3, -1), q.ap(1, 3), vv.ap(5, 3, -1), MULT)
    v.tensor_tensor(TP.ap(6, 2), q.ap(2, 2), vv.ap(3, 2), MULT)
    v.tensor_tensor(TP.ap(8), q.ap(0), vv.ap(5), MULT)
    v.tensor_tensor(TP.ap(9, 2), q.ap(0, 2, 3), vv.ap(4, 2), MULT)
    v.tensor_tensor(TP.ap(12), q.ap(1), vv.ap(3), MULT)
    v.tensor_tensor(AS.ap(0), TP.ap(0), TP.ap(1), ADD)
    v.tensor_tensor(AS.ap(1), TP.ap(3), TP.ap(5), ADD)
    v.tensor_tensor(AS.ap(2, 2), TP.ap(6, 2, 3), TP.ap(7, 2, 3), ADD)
    v.tensor_tensor(S.ap(0), AS.ap(0), TP.ap(2), ADD)
    v.tensor_tensor(S.ap(1, 3), AS.ap(1, 3), TP.ap(4, 3, 4), SUB)
    scr.free("TP", "AS")

    # quad products
    QD4 = scr.alloc("QD4", 4)   # [yy, zz, xx, yy]
    P1 = scr.alloc("P1", 3)     # [xy, xz, yz]
    P2 = scr.alloc("P2", 3)     # [zw, yw, xw]
    a.activation(QD4.ap(0, 2), q.ap(1, 2), AF.Square)              # yy, zz
    a.activation(QD4.ap(2), q.ap(0), AF.Square)                    # xx
    a.activation(QD4.ap(3), q.ap(1), AF.Square)                    # yy copy
    v.tensor_tensor(P1.ap(0, 2), q.bcast(0, 2), q.ap(1, 2), MULT)  # xy, xz
    v.tensor_tensor(P1.ap(2), q.ap(1), q.ap(2), MULT)              # yz
    v.tensor_tensor(P2.ap(0, 2), q.bcast(3, 2), q.ap(2, 2, -1), MULT)  # zw, yw
    v.tensor_tensor(P2.ap(2), q.ap(0), q.ap(3), MULT)              # xw

    QO = scr.alloc("QO", 6)     # [Qo10, Qo02, Qo21, Qo01, Qo20, Qo12]
    QDG = scr.alloc("QDG", 3)   # (yy+zz, zz+xx, xx+yy)
    v.tensor_tensor(QO.ap(0, 3), P1.ap(0, 3), P2.ap(0, 3), ADD)
    v.tensor_tensor(QO.ap(3, 3), P1.ap(0, 3), P2.ap(0, 3), SUB)
    v.tensor_tensor(QDG.ap(0, 3), QD4.ap(0, 3), QD4.ap(1, 3), ADD)
    scr.free("QD4", "P1", "P2")

    # pDot_lin = vl + 2*(Q @ vl)
    RD = scr.alloc("RD", 3)
    RO = scr.alloc("RO", 6)     # [R01, R02, R10, R12, R20, R21]
    T1 = scr.alloc("T1", 3)
    v.tensor_tensor(RD.ap(0, 3), QDG.ap(0, 3), vv.ap(0, 3), MULT)
    v.tensor_tensor(RO.ap(2, 2, -1), QO.ap(0, 2), vv.ap(0, 2, 2), MULT)
    v.tensor_tensor(RO.ap(0, 2, 4), QO.ap(3, 2), vv.ap(1, 2, -1), MULT)
    v.tensor_tensor(RO.ap(5), QO.ap(2), vv.ap(1), MULT)            # R21
    v.tensor_tensor(RO.ap(3), QO.ap(5), vv.ap(2), MULT)            # R12
    v.tensor_tensor(T1.ap(0, 3), RO.ap(0, 3, 2), RO.ap(1, 3, 2), ADD)
    v.tensor_tensor(T1.ap(0, 3), T1.ap(0, 3), RD.ap(0, 3), SUB)
    v.scalar_tensor_tensor(PL.ap(0, 3), T1.ap(0, 3), 2.0, vv.ap(0, 3),
                           MULT, ADD)
    scr.free("RD", "RO", "T1")

    # quadratic damping
    AB = scr.alloc("AB", 6)
    DVQ = scr.alloc("DVQ", 6)
    for i in range(6):
        a.activation(AB.ap(i), vv.ap(i), AF.Abs, scale=float(qd[i]))
    v.tensor_tensor(DVQ.ap(0, 6), AB.ap(0, 6), vv.ap(0, 6), MULT)
    scr.free("AB")

    # coriolis cross product va x vl
    PA = scr.alloc("PA", 3)
    PB = scr.alloc("PB", 3)
    CR = scr.alloc("CR", 3)
    v.tensor_tensor(PA.ap(0, 2), vv.ap(4, 2), vv.ap(2, 2, -2), MULT)
    v.tensor_tensor(PA.ap(2), vv.ap(3), vv.ap(1), MULT)
    v.tensor_tensor(PB.ap(1, 2), vv.ap(3, 2), vv.ap(2, 2, -2), MULT)
    v.tensor_tensor(PB.ap(0), vv.ap(5), vv.ap(1), MULT)
    v.tensor_tensor(CR.ap(0, 3), PA.ap(0, 3), PB.ap(0, 3), SUB)
    scr.free("PA", "PB")

    # rhs = u - Cv - Dv - g
    for i in range(6):
        v.scalar_tensor_tensor(RHS.ap(i), vv.ap(i), float(ld[i, i]), uu.ap(i),
                               MULT, ADD)
    for i in range(6):
        for j in range(6):
            if i != j and ld[i, j] != 0.0:
                v.scalar_tensor_tensor(RHS.ap(i), vv.ap(j), float(ld[i, j]),
                                       RHS.ap(i), MULT, ADD)
    neg = [i for i in range(6) if qd[i] < 0]
    pos = [i for i in range(6) if qd[i] >= 0 and qd[i] != 0.0]
    for grp, op in ((neg, SUB), (pos, ADD)):
        for i0, n in _runs(grp):
            v.tensor_tensor(RHS.ap(i0, n), RHS.ap(i0, n), DVQ.ap(i0 - 0, n)
                            if False else DVQ.ap(i0, n), op)
    scr.free("DVQ")
    v.scalar_tensor_tensor(RHS.ap(0, 3), CR.ap(0, 3), -m1, RHS.ap(0, 3),
                           MULT, ADD)
    scr.free("CR")

    # restoring: rot2 = (2*Qo20, 2*Qo21, 1-2*Qd2); rhs += [c1*rot2; c2 x rot2]
    TMPG = scr.alloc("TMPG", 1)
    v.scalar_tensor_tensor(RHS.ap(0, 2), QO.ap(4, 2, -2), 2.0 * c1,
                           RHS.ap(0, 2), MULT, ADD)
    v.tensor_scalar(TMPG.ap(0), QDG.ap(2), -2.0 * c1, c1, MULT, ADD)
    v.tensor_tensor(RHS.ap(2), RHS.ap(2), TMPG.ap(0), ADD)

    c2x, c2y, c2z = (float(c2[0]), float(c2[1]), float(c2[2]))
    gterms = [
        (3, [(QO, 2, -2.0 * c2z), (QDG, 2, -2.0 * c2y)], c2y),
        (4, [(QO, 4, 2.0 * c2z), (QDG, 2, 2.0 * c2x)], -c2x),
        (5, [(QO, 2, 2.0 * c2x), (QO, 4, -2.0 * c2y)], 0.0),
    ]
    for row, terms, const in gterms:
        terms = [(reg, s, co) for (reg, s, co) in terms if co != 0.0]
        if const != 0.0:
            if terms:
                reg, s, co = terms.pop(0)
                v.tensor_scalar(TMPG.ap(0), reg.ap(s), co, const, MULT, ADD)
                v.tensor_tensor(RHS.ap(row), RHS.ap(row), TMPG.ap(0), ADD)
            else:
                v.tensor_scalar(RHS.ap(row), RHS.ap(row), const, None, ADD)
        for reg, s, co in terms:
            v.scalar_tensor_tensor(RHS.ap(row), reg.ap(s), co, RHS.ap(row),
                                   MULT, ADD)
    scr.free("TMPG", "QO", "QDG")


def _runs(idxs):
    out = []
    for i in idxs:
        if out and i == out[-1][0] + out[-1][1]:
            out[-1] = (out[-1][0], out[-1][1] + 1)
        else:
            out.append((i, 1))
    return out


_CACHE = {}


def kernel(**inputs):
    from concourse.bass_utils import run_bass_kernel_spmd

    x = np.ascontiguousarray(np.asarray(inputs["x"], np.float32))
    u = np.ascontiguousarray(np.asarray(inputs["u"], np.float32))
    K = x.shape[0]
    assert K % N_CORES == 0
    K_core = K // N_CORES
    W = 256
    assert K_core % (P * W) == 0

    pp = _extract_params(inputs)
    pp_key = (K_core, W, pp["m1"], pp["c1"], tuple(pp["minv"]),
              tuple(pp["qd"]), tuple(pp["c2"]), pp["ld"].tobytes())
    if pp_key not in _CACHE:
        _CACHE[pp_key] = build_program(pp, K_core, W)
    nc = _CACHE[pp_key]

    in_maps = []
    for k in range(N_CORES):
        sl = slice(k * K_core, (k + 1) * K_core)
        in_maps.append({"x": x[sl], "u": u[sl]})

    kwargs = dict(trace=True) if TRACE else {}
    res = run_bass_kernel_spmd(nc, in_maps, core_ids=list(range(N_CORES)),
                               **kwargs)
    LAST_RUN_INFO.clear()
    LAST_RUN_INFO.update(dict(
        exec_time_ns=res.exec_time_ns,
        mean_exec_time_ns=res.mean_exec_time_ns,
        profile_json=res.profile_json,
    ))
    out = np.empty((K, 13), np.float32)
    for k in range(N_CORES):
        out[k * K_core:(k + 1) * K_core] = res.results[k]["o"]
    return out
